# revision 15
# baseline (speedup 1.0000x reference)
"""Gaussian label-splat density kernel for Trainium2 (8 NeuronCores).

Math (matches the reference): for each batch b
    gx[n, w] = exp(-(w - lx[n])^2 / (2 sigma^2))   (normalized over w)
    gy[n, h] = exp(-(h - ly[n])^2 / (2 sigma^2))   (normalized over h)
    density[b, 0] = sum_n outer(gy[n], gx[n]) = gy.T @ gx    (K = 64 labels)

batch_images contributes only its shape, so the kernel never touches it.

Sharding: core c -> (batch b = c // 2, row half t = c % 2, h0 = 256 * t).
Each core builds its own gaussians from a 2 KB label packet and emits a
(256, 512) output tile as two 128x512 matmuls. No cross-core comms.

The x profile is materialized in full (it is the matmul rhs) and Zx is a
row-sum of it. The y profile is only ever needed through its normalizer
Zy and a 256-row slice: Zy is computed WITHOUT the full-range pass via
the exact split  sum_{h=0..511} = sum_{h in Z} - left tail - right tail,
where the infinite sum is sigma*sqrt(2*pi) (Poisson summation; the theta
correction is < 3e-9 for sigma >= 1) and both 64-term tails fit in one
small (64,128) exp. Both normalizers (1/Zx * 1/Zy) fold into the small
y-slice (lhsT); matmuls run in f32r (single PE pass). An
input-independent warm-up exp pulls the ~1.3us ACT table load into the
label-DMA wait window. The store path (PSUM->SBUF copies + output DMAs)
stays on Vector + Sync, which wake from semaphore waits in ~30ns (Scalar
pays a ~600ns wake lag after idling).

Label packet (built on host), partitions 0..63 = labels:
    col 0 = -lx          (bias for the x square)
    col 1 = h0 - ly      (bias for the y row-window square)
    col 2 = ly + 1       (left-tail offset)
    col 3 = 512 - ly     (right-tail offset)
    col 4 = sigma
"""

import numpy as np

import concourse.bacc as bacc
import concourse.tile as tile
from concourse.tile import add_dep_helper
from concourse import mybir
from concourse.bass_utils import run_bass_kernel_spmd

B, NLAB, H, W = 4, 64, 512, 512
P = 128
HALF = H // 2  # output rows per core
NTAIL = 64  # terms per truncation tail
N_CORES = 8
F32 = mybir.dt.float32
F32R = mybir.dt.float32r
SQRT_2PI = 2.5066282746310002

_CACHE: list = []


def _build():
    AF = mybir.ActivationFunctionType
    AX = mybir.AxisListType
    OP = mybir.AluOpType
    nc = bacc.Bacc(
        "TRN2",
        debug=False,
        target_bir_lowering=False,
        num_devices=N_CORES,
        enable_partition_id=False,
    )
    labels = nc.dram_tensor("labels", (NLAB, 8), F32, kind="ExternalInput").ap()
    out = nc.dram_tensor("out", (HALF, W), F32, kind="ExternalOutput").ap()

    with tile.TileContext(nc) as tc:
        with (
            tc.tile_pool(name="sb", bufs=1) as pool,
            tc.tile_pool(name="ob", bufs=2) as opool,
            tc.tile_pool(name="ps", bufs=2, space="PSUM") as psum,
        ):
            # input-independent warm-up op so walrus's ACT_TABLE_LOAD lands
            # here and hides under the label DMA's completion latency
            warm = pool.tile([NLAB, 1], F32)
            nc.vector.memset(warm, 0.0)
            nc.scalar.activation(warm, warm, AF.Exp, scale=1.0)

            L = pool.tile([NLAB, 8], F32)
            nc.sync.dma_start(out=L, in_=labels)

            I = pool.tile([NLAB, W], F32)
            nc.gpsimd.iota(
                I,
                pattern=[[1, W]],
                base=0,
                channel_multiplier=0,
                allow_small_or_imprecise_dtypes=True,
            )

            # M = -1 / (2 sigma^2): (sigma * sigma) * -2, then reciprocal
            s2n = pool.tile([NLAB, 1], F32)
            nc.vector.tensor_scalar(s2n, L[:, 4:5], L[:, 4:5], -2.0, OP.mult, OP.mult)
            M = pool.tile([NLAB, 1], F32)
            nc.vector.reciprocal(M, s2n)
            # sigma * sqrt(2 pi) = the infinite-range gaussian sum
            Zfull = pool.tile([NLAB, 1], F32)
            nc.vector.tensor_scalar_mul(Zfull, L[:, 4:5], SQRT_2PI)

            # x square on ACT, then the full x profile (matmul rhs, f32r)
            SQx = pool.tile([NLAB, W], F32)
            i_sqx = nc.scalar.activation(SQx, I, AF.Square, bias=L[:, 0:1], scale=1.0)
            Gx = pool.tile([NLAB, W], F32R)
            i_ex = nc.scalar.activation(Gx, SQx, AF.Exp, scale=M)
            Zx = pool.tile([NLAB, 1], F32)
            nc.vector.reduce_sum(Zx, Gx, axis=AX.X)
            Rx = pool.tile([NLAB, 1], F32)
            nc.vector.reciprocal(Rx, Zx)

            # y truncation tails: cols 0..63 = j + (ly+1), 64..127 = j + (512-ly)
            Dt = pool.tile([NLAB, 2 * NTAIL], F32)
            nc.vector.tensor_scalar_add(Dt[:, 0:NTAIL], I[:, 0:NTAIL], L[:, 2:3])
            nc.vector.tensor_scalar_add(
                Dt[:, NTAIL : 2 * NTAIL], I[:, 0:NTAIL], L[:, 3:4]
            )
            SQt = pool.tile([NLAB, 2 * NTAIL], F32)
            nc.vector.tensor_mul(SQt, Dt, Dt)
            Gt = pool.tile([NLAB, 2 * NTAIL], F32)
            Tsum = pool.tile([NLAB, 1], F32)
            i_et = nc.scalar.activation(Gt, SQt, AF.Exp, scale=M, accum_out=Tsum)
            # the subtract runs on the otherwise-idle GpSimd so the Vector
            # queue (row-sum -> reciprocals -> normalize) stays short
            Zy = pool.tile([NLAB, 1], F32)
            nc.gpsimd.tensor_sub(Zy, Zfull, Tsum)

            # y slice square (DVE) + exp (ACT)
            Ds = pool.tile([NLAB, HALF], F32)
            nc.vector.tensor_scalar_add(Ds, I[:, 0:HALF], L[:, 1:2])
            SQs = pool.tile([NLAB, HALF], F32)
            nc.vector.tensor_mul(SQs, Ds, Ds)
            Gs = pool.tile([NLAB, HALF], F32)
            i_es = nc.scalar.activation(Gs, SQs, AF.Exp, scale=M)
            # pin the ACT queue order: SQx -> Ex -> tails-exp -> slice-exp, so
            # the x chain (which feeds the long DVE row-sum) never slips
            add_dep_helper(i_et.ins, i_ex.ins, sync=False, reason="ACT order: tails after Ex")
            add_dep_helper(i_es.ins, i_et.ins, sync=False, reason="ACT order: slice last")

            Ry = pool.tile([NLAB, 1], F32)
            nc.vector.reciprocal(Ry, Zy)

            # both normalizers fold into the small lhsT in one dual-scalar op
            # per half (no separate Rx*Ry product); rhs = Gx raw. Halved so
            # the first LDWEIGHTS can start sooner.
            GYn = pool.tile([NLAB, HALF], F32R)
            nc.vector.tensor_scalar(
                GYn[:, 0:P], Gs[:, 0:P], Rx, Ry, OP.mult, OP.mult
            )
            nc.vector.tensor_scalar(
                GYn[:, P:HALF], Gs[:, P:HALF], Rx, Ry, OP.mult, OP.mult
            )

            for t in range(2):
                acc = psum.tile([P, W], F32)
                nc.tensor.matmul(
                    acc,
                    GYn[:, t * P : (t + 1) * P],
                    Gx,
                    start=True,
                    stop=True,
                )
                Ot = opool.tile([P, W], F32)
                # Scalar has a ~600ns semaphore-wake lag after idling, so the
                # whole store path stays on Vector (copies) + Sync (DMAs),
                # which wake in ~30ns
                nc.vector.tensor_copy(Ot, acc)
                nc.sync.dma_start(out=out[t * P : (t + 1) * P, :], in_=Ot)

    nc.compile()
    return nc


def _in_maps(batch_labels: np.ndarray, sigma: float) -> list:
    maps = []
    for c in range(N_CORES):
        b, t = divmod(c, 2)
        h0 = t * HALF
        lx = batch_labels[b, :, 0]
        ly = batch_labels[b, :, 1]
        packed = np.zeros((NLAB, 8), np.float32)
        packed[:, 0] = -lx
        packed[:, 1] = h0 - ly
        packed[:, 2] = ly + 1.0
        packed[:, 3] = float(H) - ly
        packed[:, 4] = sigma
        maps.append({"labels": packed})
    return maps


def _get_nc():
    if not _CACHE:
        _CACHE.append(_build())
    return _CACHE[0]


def _gather(results) -> np.ndarray:
    density = np.empty((B, 1, H, W), np.float32)
    for c in range(N_CORES):
        b, t = divmod(c, 2)
        density[b, 0, t * HALF : (t + 1) * HALF, :] = results[c]["out"]
    return density


def kernel(batch_images, batch_labels, sigma) -> np.ndarray:
    batch_labels = np.asarray(batch_labels, dtype=np.float32)
    sigma = float(np.asarray(sigma))
    nc = _get_nc()
    res = run_bass_kernel_spmd(
        nc, _in_maps(batch_labels, sigma), core_ids=list(range(N_CORES))
    )
    return _gather(res.results)
